# revision 1
# baseline (speedup 1.0000x reference)
"""Trainium2 Bass kernel for nn_BackboneBuilder_28286654611922.

The reference builds protein-backbone coordinates with a NeRF recurrence:

    out = p3 + r * (st*cp*m + st*sp*n - ct*bc)

where n = normalize(cross(p2-p1, bc)) and m = cross(n, bc).

Key structural fact (holds in exact IEEE arithmetic, any platform): the
initial residue N0=(0,0,0), CA0=(1.458,0,0), C0=(2.983,0,0) is collinear
on the x-axis.  Every cross product of x-axis vectors is exactly zero
(each component is a product with an exact-zero factor), so n = m = 0
for every placement, each new atom is p3 - r*ct*bc (still on the x-axis),
and by induction the whole trajectory stays on the x-axis with y = z = 0
exactly.  The torsion inputs phi/psi/omega enter only through cp/sp,
which multiply the zero vectors m and n — the output is therefore
INDEPENDENT of the inputs and identical across the batch.

The whole problem collapses to: broadcast a fixed fp32 table of four
512-long x-coordinate sequences (N, CA, C, O) into four [2048, 512, 3]
outputs.  That makes the kernel purely memory-bound: each of the 8
NeuronCores writes its 256-row batch shard (6.29 MB) to HBM.

Device kernel (per core, raw Bass, no Tile framework):
  - input  "tbl" [32, 6144]: the 24 KB atom-row table replicated on 32
    SBUF partitions with stride 4 (partitions {0,4,...,124}), which maps
    onto all 16 SDMA engines (even engines serve partitions 0-63, odd
    engines 64-127, in blocks of 4).
  - one 768 KB DMA loads it into SBUF (sync/HWDGE ring).
  - four output DMAs (two on the sync ring, two on the scalar ring, so
    both HWDGE rings run in parallel) each write one atom's [256, 1536]
    shard using a stride-0 inner broadcast: each SBUF partition line is
    written to 8 output rows.
This measures ~31 us/core on HW: ~11 us fixed bass preamble/tail plus
~20 us of DMA at the ~360 GB/s per-core HBM roofline.
"""

import math

import numpy as np

B, N = 2048, 512
NCORES = 8
ROWS = B // NCORES  # 256
FREE = N * 3  # 1536

_N_CA_LEN, _CA_C_LEN, _C_O_LEN, _C_N_LEN = 1.458, 1.525, 1.231, 1.329
_EPS = 1e-8


def _nerf(p1, p2, p3, r, theta, phi):
    """fp32 replica of the reference _nerf for a single chain [3]-vectors."""
    dt = np.float32
    bc = p3 - p2
    bc = bc / (np.sqrt(np.sum(bc * bc, dtype=dt), dtype=dt) + dt(_EPS))
    n = np.cross(p2 - p1, bc).astype(dt)
    n = n / (np.sqrt(np.sum(n * n, dtype=dt), dtype=dt) + dt(_EPS))
    m = np.cross(n, bc).astype(dt)
    st, ct = dt(math.sin(theta)), dt(math.cos(theta))
    cp = np.cos(phi, dtype=dt)
    sp = np.sin(phi, dtype=dt)
    return p3 + dt(r) * (st * cp * m + st * sp * n - ct * bc)


def build_table():
    """The (input-independent) backbone trajectory, fp32, shape [4, 512, 3]."""
    dt = np.float32
    n_ca_c = math.radians(111.0)
    ca_c_n = math.radians(116.5)
    ca_c_o = math.radians(120.8)
    c_n_ca = math.radians(121.7)
    zero = dt(0.0)

    N0 = np.zeros(3, dt)
    CA0 = np.array([_N_CA_LEN, 0.0, 0.0], dt)
    C0 = CA0 + np.array([_CA_C_LEN, 0.0, 0.0], dt)
    # psi[:,0] + pi only feeds cp/sp, which multiply exact-zero vectors.
    O0 = _nerf(CA0, CA0, C0, _C_O_LEN, ca_c_o, zero)
    cn_off = np.array([_C_N_LEN, 0.0, 0.0], dt)
    Np, CAp, Cp = N0, CA0, C0
    Ns, CAs, Cs, Os = [N0], [CA0], [C0], [O0]
    for i in range(1, N):
        Ni = (Cp + cn_off) if i == 1 else _nerf(CAp, Cp, Np, _C_N_LEN, ca_c_n, zero)
        p3_ca = Cp if i == 1 else CAp
        CAi = _nerf(Cp, Ni, p3_ca, _N_CA_LEN, c_n_ca, zero)
        Ci = _nerf(Ni, CAi, Ni, _CA_C_LEN, n_ca_c, zero)
        Oi = _nerf(Ni, CAi, Ci, _C_O_LEN, ca_c_o, zero)
        Np, CAp, Cp = Ni, CAi, Ci
        Ns.append(Ni)
        CAs.append(CAi)
        Cs.append(Ci)
        Os.append(Oi)
    return np.stack([np.stack(Ns), np.stack(CAs), np.stack(Cs), np.stack(Os)], 0)


def _build_bass():
    import concourse.bass as bass
    import concourse.mybir as mybir

    nc = bass.Bass(enable_partition_id=False, monotonic_sem_count=0)
    tbl = nc.declare_dram_parameter("tbl", [32, 4 * FREE], mybir.dt.float32, isOutput=False)
    outs = [
        nc.declare_dram_parameter(f"out{a}", [ROWS, FREE], mybir.dt.float32, isOutput=True)
        for a in range(4)
    ]
    with (
        nc.sbuf_tensor([128, 4 * FREE], mybir.dt.float32) as tile,
        nc.semaphore("in_sem") as in_sem,
        nc.semaphore("out_sem") as out_sem,
        nc.Block() as block,
    ):

        def emit_out(eng, a):
            # partition line -> 8 consecutive output rows (stride-0 repeat)
            src = tile[0:128:4, a * FREE : (a + 1) * FREE].unsqueeze(1).broadcast_to(
                [32, 8, FREE]
            )
            dst = outs[a][:, :].rearrange("(j k) f -> j k f", j=32)
            eng.dma_start(out=dst, in_=src).then_inc(out_sem, 16)

        @block.sync
        def _(sync):
            sync.dma_start(out=tile[0:128:4, :], in_=tbl[:, :]).then_inc(in_sem, 16)
            sync.wait_ge(in_sem, 16)
            emit_out(sync, 0)
            emit_out(sync, 1)
            sync.wait_ge(out_sem, 64)

        @block.scalar
        def _(scalar):
            scalar.wait_ge(in_sem, 16)
            emit_out(scalar, 2)
            emit_out(scalar, 3)
            scalar.wait_ge(out_sem, 64)
    return nc


_CACHE = {}


def _get_compiled():
    if "nc" not in _CACHE:
        table = build_table()  # [4, 512, 3]
        row = np.ascontiguousarray(table.reshape(4 * FREE))
        in_arr = np.ascontiguousarray(np.broadcast_to(row[None, :], (32, 4 * FREE)))
        _CACHE["table"] = table
        _CACHE["in_arr"] = in_arr
        _CACHE["nc"] = _build_bass()
    return _CACHE["nc"], _CACHE["in_arr"], _CACHE["table"]


def run_on_device(trace=False):
    from concourse.bass_utils import run_bass_kernel_spmd

    nc, in_arr, _ = _get_compiled()
    in_maps = [{"tbl": in_arr} for _ in range(NCORES)]
    return run_bass_kernel_spmd(nc, in_maps, list(range(NCORES)), trace=trace)


def kernel(phi, psi, omega):
    assert phi.shape == (B, N) and psi.shape == (B, N) and omega.shape == (B, N)
    r = run_on_device(trace=False)
    full = []
    for a in range(4):
        shards = [
            np.asarray(r.results[c][f"out{a}"]).reshape(ROWS, N, 3) for c in range(NCORES)
        ]
        full.append(np.ascontiguousarray(np.concatenate(shards, axis=0), dtype=np.float32))
    return tuple(full)  # (N, CA, C, O), each [2048, 512, 3] float32


# revision 2
# speedup vs baseline: 1.0373x; 1.0373x over previous
"""Trainium2 Bass kernel for nn_BackboneBuilder_28286654611922.

The reference builds protein-backbone coordinates with a NeRF recurrence:

    out = p3 + r * (st*cp*m + st*sp*n - ct*bc)

where n = normalize(cross(p2-p1, bc)) and m = cross(n, bc).

Key structural fact (holds in exact IEEE arithmetic, any platform): the
initial residue N0=(0,0,0), CA0=(1.458,0,0), C0=(2.983,0,0) is collinear
on the x-axis.  Every cross product of x-axis vectors is exactly zero
(each component is a product with an exact-zero factor), so n = m = 0
for every placement, each new atom is p3 - r*ct*bc (still on the x-axis),
and by induction the whole trajectory stays on the x-axis with y = z = 0
exactly.  The torsion inputs phi/psi/omega enter only through cp/sp,
which multiply the zero vectors m and n — the output is therefore
INDEPENDENT of the inputs and identical across the batch.

The whole problem collapses to: broadcast a fixed fp32 table of four
512-long x-coordinate sequences (N, CA, C, O) into four [2048, 512, 3]
outputs.  That makes the kernel purely memory-bound: each of the 8
NeuronCores writes its 256-row batch shard (6.29 MB) to HBM.

Device kernel (per core, raw Bass, no Tile framework):
  - input  "tbl" [32, 6144]: the 24 KB atom-row table replicated on 32
    SBUF partitions with stride 4 (partitions {0,4,...,124}), which maps
    onto all 16 SDMA engines (even engines serve partitions 0-63, odd
    engines 64-127, in blocks of 4).
  - one 768 KB DMA loads it into SBUF (sync/HWDGE ring).
  - four output DMAs (two on the sync ring, two on the scalar ring, so
    both HWDGE rings run in parallel) each write one atom's [256, 1536]
    shard using a stride-0 inner broadcast: each SBUF partition line is
    written to 8 output rows.
This measures ~31 us/core on HW: ~11 us fixed bass preamble/tail plus
~20 us of DMA at the ~360 GB/s per-core HBM roofline.
"""

import math

import numpy as np

B, N = 2048, 512
NCORES = 8
ROWS = B // NCORES  # 256
FREE = N * 3  # 1536

_N_CA_LEN, _CA_C_LEN, _C_O_LEN, _C_N_LEN = 1.458, 1.525, 1.231, 1.329
_EPS = 1e-8


def _nerf(p1, p2, p3, r, theta, phi):
    """fp32 replica of the reference _nerf for a single chain [3]-vectors."""
    dt = np.float32
    bc = p3 - p2
    bc = bc / (np.sqrt(np.sum(bc * bc, dtype=dt), dtype=dt) + dt(_EPS))
    n = np.cross(p2 - p1, bc).astype(dt)
    n = n / (np.sqrt(np.sum(n * n, dtype=dt), dtype=dt) + dt(_EPS))
    m = np.cross(n, bc).astype(dt)
    st, ct = dt(math.sin(theta)), dt(math.cos(theta))
    cp = np.cos(phi, dtype=dt)
    sp = np.sin(phi, dtype=dt)
    return p3 + dt(r) * (st * cp * m + st * sp * n - ct * bc)


def build_table():
    """The (input-independent) backbone trajectory, fp32, shape [4, 512, 3]."""
    dt = np.float32
    n_ca_c = math.radians(111.0)
    ca_c_n = math.radians(116.5)
    ca_c_o = math.radians(120.8)
    c_n_ca = math.radians(121.7)
    zero = dt(0.0)

    N0 = np.zeros(3, dt)
    CA0 = np.array([_N_CA_LEN, 0.0, 0.0], dt)
    C0 = CA0 + np.array([_CA_C_LEN, 0.0, 0.0], dt)
    # psi[:,0] + pi only feeds cp/sp, which multiply exact-zero vectors.
    O0 = _nerf(CA0, CA0, C0, _C_O_LEN, ca_c_o, zero)
    cn_off = np.array([_C_N_LEN, 0.0, 0.0], dt)
    Np, CAp, Cp = N0, CA0, C0
    Ns, CAs, Cs, Os = [N0], [CA0], [C0], [O0]
    for i in range(1, N):
        Ni = (Cp + cn_off) if i == 1 else _nerf(CAp, Cp, Np, _C_N_LEN, ca_c_n, zero)
        p3_ca = Cp if i == 1 else CAp
        CAi = _nerf(Cp, Ni, p3_ca, _N_CA_LEN, c_n_ca, zero)
        Ci = _nerf(Ni, CAi, Ni, _CA_C_LEN, n_ca_c, zero)
        Oi = _nerf(Ni, CAi, Ci, _C_O_LEN, ca_c_o, zero)
        Np, CAp, Cp = Ni, CAi, Ci
        Ns.append(Ni)
        CAs.append(CAi)
        Cs.append(Ci)
        Os.append(Oi)
    return np.stack([np.stack(Ns), np.stack(CAs), np.stack(Cs), np.stack(Os)], 0)


def _build_bass():
    import concourse.bass as bass
    import concourse.mybir as mybir

    nc = bass.Bass(enable_partition_id=False, monotonic_sem_count=0)
    tbl = nc.declare_dram_parameter("tbl", [32, 4 * FREE], mybir.dt.float32, isOutput=False)
    outs = [
        nc.declare_dram_parameter(f"out{a}", [ROWS, FREE], mybir.dt.float32, isOutput=True)
        for a in range(4)
    ]
    with (
        nc.sbuf_tensor([128, 4 * FREE], mybir.dt.float32) as tile,
        nc.semaphore("semA") as semA,
        nc.semaphore("semB") as semB,
        nc.semaphore("out_sem") as out_sem,
        nc.Block() as block,
    ):

        def emit_out(eng, a):
            # partition line -> 8 consecutive output rows (stride-0 repeat)
            src = tile[0:128:4, a * FREE : (a + 1) * FREE].unsqueeze(1).broadcast_to(
                [32, 8, FREE]
            )
            dst = outs[a][:, :].rearrange("(j k) f -> j k f", j=32)
            eng.dma_start(out=dst, in_=src).then_inc(out_sem, 16)

        @block.sync
        def _(sync):
            sync.dma_start(
                out=tile[0:128:4, 0 : 2 * FREE], in_=tbl[:, 0 : 2 * FREE]
            ).then_inc(semA, 16)
            sync.wait_ge(semA, 16)
            emit_out(sync, 0)
            emit_out(sync, 1)
            sync.wait_ge(out_sem, 64)

        @block.scalar
        def _(scalar):
            scalar.dma_start(
                out=tile[0:128:4, 2 * FREE : 4 * FREE], in_=tbl[:, 2 * FREE : 4 * FREE]
            ).then_inc(semB, 16)
            scalar.wait_ge(semB, 16)
            emit_out(scalar, 2)
            emit_out(scalar, 3)
            scalar.wait_ge(out_sem, 64)
    return nc


_CACHE = {}


def _get_compiled():
    if "nc" not in _CACHE:
        table = build_table()  # [4, 512, 3]
        row = np.ascontiguousarray(table.reshape(4 * FREE))
        in_arr = np.ascontiguousarray(np.broadcast_to(row[None, :], (32, 4 * FREE)))
        _CACHE["table"] = table
        _CACHE["in_arr"] = in_arr
        _CACHE["nc"] = _build_bass()
    return _CACHE["nc"], _CACHE["in_arr"], _CACHE["table"]


def run_on_device(trace=False):
    from concourse.bass_utils import run_bass_kernel_spmd

    nc, in_arr, _ = _get_compiled()
    in_maps = [{"tbl": in_arr} for _ in range(NCORES)]
    return run_bass_kernel_spmd(nc, in_maps, list(range(NCORES)), trace=trace)


def kernel(phi, psi, omega):
    assert phi.shape == (B, N) and psi.shape == (B, N) and omega.shape == (B, N)
    r = run_on_device(trace=False)
    full = []
    for a in range(4):
        shards = [
            np.asarray(r.results[c][f"out{a}"]).reshape(ROWS, N, 3) for c in range(NCORES)
        ]
        full.append(np.ascontiguousarray(np.concatenate(shards, axis=0), dtype=np.float32))
    return tuple(full)  # (N, CA, C, O), each [2048, 512, 3] float32


# revision 3
# speedup vs baseline: 1.0599x; 1.0218x over previous
"""Trainium2 Bass kernel for nn_BackboneBuilder_28286654611922.

The reference builds protein-backbone coordinates with a NeRF recurrence:

    out = p3 + r * (st*cp*m + st*sp*n - ct*bc)

where n = normalize(cross(p2-p1, bc)) and m = cross(n, bc).

Key structural fact (holds in exact IEEE arithmetic, any platform): the
initial residue N0=(0,0,0), CA0=(1.458,0,0), C0=(2.983,0,0) is collinear
on the x-axis.  Every cross product of x-axis vectors is exactly zero
(each component is a product with an exact-zero factor), so n = m = 0
for every placement, each new atom is p3 - r*ct*bc (still on the x-axis),
and by induction the whole trajectory stays on the x-axis with y = z = 0
exactly.  The torsion inputs phi/psi/omega enter only through cp/sp,
which multiply the zero vectors m and n — the output is therefore
INDEPENDENT of the inputs and identical across the batch.

The whole problem collapses to: broadcast a fixed fp32 table of four
512-long x-coordinate sequences (N, CA, C, O) into four [2048, 512, 3]
outputs.  That makes the kernel purely memory-bound: each of the 8
NeuronCores writes its 256-row batch shard (6.29 MB) to HBM.

Device kernel (per core, raw Bass, no Tile framework):
  - input  "tbl" [32, 6144]: the 24 KB atom-row table replicated on 32
    SBUF partitions with stride 4 (partitions {0,4,...,124}), which maps
    onto all 16 SDMA engines (even engines serve partitions 0-63, odd
    engines 64-127, in blocks of 4).
  - one 768 KB DMA loads it into SBUF (sync/HWDGE ring).
  - four output DMAs (two on the sync ring, two on the scalar ring, so
    both HWDGE rings run in parallel) each write one atom's [256, 1536]
    shard using a stride-0 inner broadcast: each SBUF partition line is
    written to 8 output rows.
This measures ~31 us/core on HW: ~11 us fixed bass preamble/tail plus
~20 us of DMA at the ~360 GB/s per-core HBM roofline.
"""

import math

import numpy as np

B, N = 2048, 512
NCORES = 8
ROWS = B // NCORES  # 256
FREE = N * 3  # 1536

_N_CA_LEN, _CA_C_LEN, _C_O_LEN, _C_N_LEN = 1.458, 1.525, 1.231, 1.329
_EPS = 1e-8


def _nerf(p1, p2, p3, r, theta, phi):
    """fp32 replica of the reference _nerf for a single chain [3]-vectors."""
    dt = np.float32
    bc = p3 - p2
    bc = bc / (np.sqrt(np.sum(bc * bc, dtype=dt), dtype=dt) + dt(_EPS))
    n = np.cross(p2 - p1, bc).astype(dt)
    n = n / (np.sqrt(np.sum(n * n, dtype=dt), dtype=dt) + dt(_EPS))
    m = np.cross(n, bc).astype(dt)
    st, ct = dt(math.sin(theta)), dt(math.cos(theta))
    cp = np.cos(phi, dtype=dt)
    sp = np.sin(phi, dtype=dt)
    return p3 + dt(r) * (st * cp * m + st * sp * n - ct * bc)


def build_table():
    """The (input-independent) backbone trajectory, fp32, shape [4, 512, 3]."""
    dt = np.float32
    n_ca_c = math.radians(111.0)
    ca_c_n = math.radians(116.5)
    ca_c_o = math.radians(120.8)
    c_n_ca = math.radians(121.7)
    zero = dt(0.0)

    N0 = np.zeros(3, dt)
    CA0 = np.array([_N_CA_LEN, 0.0, 0.0], dt)
    C0 = CA0 + np.array([_CA_C_LEN, 0.0, 0.0], dt)
    # psi[:,0] + pi only feeds cp/sp, which multiply exact-zero vectors.
    O0 = _nerf(CA0, CA0, C0, _C_O_LEN, ca_c_o, zero)
    cn_off = np.array([_C_N_LEN, 0.0, 0.0], dt)
    Np, CAp, Cp = N0, CA0, C0
    Ns, CAs, Cs, Os = [N0], [CA0], [C0], [O0]
    for i in range(1, N):
        Ni = (Cp + cn_off) if i == 1 else _nerf(CAp, Cp, Np, _C_N_LEN, ca_c_n, zero)
        p3_ca = Cp if i == 1 else CAp
        CAi = _nerf(Cp, Ni, p3_ca, _N_CA_LEN, c_n_ca, zero)
        Ci = _nerf(Ni, CAi, Ni, _CA_C_LEN, n_ca_c, zero)
        Oi = _nerf(Ni, CAi, Ci, _C_O_LEN, ca_c_o, zero)
        Np, CAp, Cp = Ni, CAi, Ci
        Ns.append(Ni)
        CAs.append(CAi)
        Cs.append(Ci)
        Os.append(Oi)
    return np.stack([np.stack(Ns), np.stack(CAs), np.stack(Cs), np.stack(Os)], 0)


def _build_bass():
    import concourse.bass as bass
    import concourse.mybir as mybir

    nc = bass.Bass(enable_partition_id=False, monotonic_sem_count=0)
    tbl = nc.declare_dram_parameter("tbl", [32, 4 * FREE], mybir.dt.float32, isOutput=False)
    outs = [
        nc.declare_dram_parameter(f"out{a}", [ROWS, FREE], mybir.dt.float32, isOutput=True)
        for a in range(4)
    ]
    with (
        nc.sbuf_tensor([128, 4 * FREE], mybir.dt.float32) as tile,
        nc.semaphore("s0") as s0,
        nc.semaphore("s1") as s1,
        nc.semaphore("s2") as s2,
        nc.semaphore("s3") as s3,
        nc.semaphore("out_sem") as out_sem,
        nc.Block() as block,
    ):
        sems = [s0, s1, s2, s3]

        def emit_in(eng, a):
            eng.dma_start(
                out=tile[0:128:4, a * FREE : (a + 1) * FREE],
                in_=tbl[:, a * FREE : (a + 1) * FREE],
            ).then_inc(sems[a], 16)

        def emit_out(eng, a):
            # partition line -> 8 consecutive output rows (stride-0 repeat)
            src = tile[0:128:4, a * FREE : (a + 1) * FREE].unsqueeze(1).broadcast_to(
                [32, 8, FREE]
            )
            dst = outs[a][:, :].rearrange("(j k) f -> j k f", j=32)
            eng.dma_start(out=dst, in_=src).then_inc(out_sem, 16)

        @block.sync
        def _(sync):
            emit_in(sync, 0)
            emit_in(sync, 1)
            sync.wait_ge(s0, 16)
            emit_out(sync, 0)
            sync.wait_ge(s1, 16)
            emit_out(sync, 1)
            sync.wait_ge(out_sem, 64)

        @block.scalar
        def _(scalar):
            emit_in(scalar, 2)
            emit_in(scalar, 3)
            scalar.wait_ge(s2, 16)
            emit_out(scalar, 2)
            scalar.wait_ge(s3, 16)
            emit_out(scalar, 3)
            scalar.wait_ge(out_sem, 64)
    return nc


_CACHE = {}


def _get_compiled():
    if "nc" not in _CACHE:
        table = build_table()  # [4, 512, 3]
        row = np.ascontiguousarray(table.reshape(4 * FREE))
        in_arr = np.ascontiguousarray(np.broadcast_to(row[None, :], (32, 4 * FREE)))
        _CACHE["table"] = table
        _CACHE["in_arr"] = in_arr
        _CACHE["nc"] = _build_bass()
    return _CACHE["nc"], _CACHE["in_arr"], _CACHE["table"]


def run_on_device(trace=False):
    from concourse.bass_utils import run_bass_kernel_spmd

    nc, in_arr, _ = _get_compiled()
    in_maps = [{"tbl": in_arr} for _ in range(NCORES)]
    return run_bass_kernel_spmd(nc, in_maps, list(range(NCORES)), trace=trace)


def kernel(phi, psi, omega):
    assert phi.shape == (B, N) and psi.shape == (B, N) and omega.shape == (B, N)
    r = run_on_device(trace=False)
    full = []
    for a in range(4):
        shards = [
            np.asarray(r.results[c][f"out{a}"]).reshape(ROWS, N, 3) for c in range(NCORES)
        ]
        full.append(np.ascontiguousarray(np.concatenate(shards, axis=0), dtype=np.float32))
    return tuple(full)  # (N, CA, C, O), each [2048, 512, 3] float32
